# revision 10
# baseline (speedup 1.0000x reference)
"""ClusterPooling (segment-mean + row gather) on 8 Trainium2 NeuronCores.

Strategy (cluster-sorted / packed layout, no cross-core communication):
  - Host packs the 50000 clusters into 392 "cluster tiles" of <=128 clusters
    each, such that every tile owns <= T*128 node rows (T=4 typically).
    Tiles are distributed 49 per core; node rows are permuted so each tile's
    nodes are contiguous and padded to exactly T*128 rows.
  - On device, each cluster tile's segment-sum is computed as a one-hot
    matmul: S[n, c] = (slot_id[n] == c) built with a DVE is_equal against an
    iota row, then PSUM accumulation of S.T @ x over the T node sub-tiles.
    Means come from a per-partition tensor_scalar multiply by 1/count.
  - pos[sample_index] is a per-partition indirect DMA gather (128 rows/call).
  - Host applies the inverse cluster permutation to assemble full outputs.
"""

import heapq
import sys
import time

import numpy as np

if "/opt/trn_rl_repo" not in sys.path:
    sys.path.insert(0, "/opt/trn_rl_repo")

import concourse.bacc as bacc
import concourse.bass as bass
import concourse.mybir as mybir
import concourse.tile as tile
from concourse.bass_utils import run_bass_kernel_spmd

P = 128
N_CORES = 8
MM_DTYPE = "f32r"  # "f32r" (fast, ~1e-4 rel err) or "f32" (exact)

_PROGRAM_CACHE = {}
_last_in_maps = None


# ----------------------------------------------------------------------------
# Host-side layout
# ----------------------------------------------------------------------------

def _pack_clusters(counts, n_bins):
    """Assign each cluster to a bin s.t. loads balance and <=128 clusters/bin.

    Returns (bin_of[C], slot_of[C], max_load).
    """
    C = counts.shape[0]
    order = np.argsort(-counts, kind="stable")
    bin_of = np.empty(C, dtype=np.int64)
    slot_of = np.empty(C, dtype=np.int64)
    heap = [(0, 0, b) for b in range(n_bins)]
    heapq.heapify(heap)
    spill = []
    for c in order:
        cnt = int(counts[c])
        while True:
            load, slots, b = heapq.heappop(heap)
            if slots < P:
                break
            spill.append((load, slots, b))
        for t in spill:
            heapq.heappush(heap, t)
        spill.clear()
        bin_of[c] = b
        slot_of[c] = slots
        heapq.heappush(heap, (load + cnt, slots + 1, b))
    loads = np.zeros(n_bins, dtype=np.int64)
    np.add.at(loads, bin_of, counts)
    return bin_of, slot_of, int(loads.max())


def _build_layout(cluster_map, sample_index, C):
    """All index bookkeeping for the packed layout. Pure host/index work."""
    N = cluster_map.shape[0]
    counts = np.bincount(cluster_map, minlength=C).astype(np.int64)

    tiles_per_core = max(1, int(np.ceil(C / (N_CORES * P))))
    n_bins = N_CORES * tiles_per_core
    bin_of, slot_of, max_load = _pack_clusters(counts, n_bins)
    T = max(1, int(np.ceil(max_load / P)))  # node sub-tiles per cluster tile
    cap = T * P

    # node permutation: order nodes by (bin, slot); pad each bin to cap rows
    key = bin_of[cluster_map] * P + slot_of[cluster_map]
    perm = np.argsort(key, kind="stable")  # nodes grouped by bin then slot
    loads = np.zeros(n_bins, dtype=np.int64)
    np.add.at(loads, bin_of, counts)
    bin_node_start = np.zeros(n_bins + 1, dtype=np.int64)
    np.cumsum(loads, out=bin_node_start[1:])

    # padded node index table: for each (bin, padded row) the source node or -1
    node_src = np.full(n_bins * cap, -1, dtype=np.int64)
    row_in_bin = np.arange(N) - bin_node_start[:-1].repeat(loads)
    dest = np.arange(N // N if False else 0)  # placeholder, replaced below
    bins_rep = np.repeat(np.arange(n_bins), loads)
    node_src[bins_rep * cap + row_in_bin] = perm
    # relmap value per padded row: slot of its cluster, or -1 for padding
    rel = np.full(n_bins * cap, -1.0, dtype=np.float32)
    rel[bins_rep * cap + row_in_bin] = slot_of[cluster_map[perm]].astype(np.float32)

    inv_counts = (1.0 / np.maximum(counts, 1)).astype(np.float32)
    # per-bin [P] inverse-count vector (1.0 for unused slots)
    invc = np.ones((n_bins, P), dtype=np.float32)
    invc[bin_of, slot_of] = inv_counts

    # sample offsets per core, padded to tiles-of-128
    S_total = sample_index.shape[0]
    s_per_core = int(np.ceil(S_total / N_CORES))
    s_tiles = int(np.ceil(s_per_core / P))
    s_pad = N_CORES * s_tiles * P
    soff_flat = np.zeros(s_pad, dtype=np.int32)
    soff_flat[:S_total] = sample_index.astype(np.int32)

    return dict(
        counts=counts, bin_of=bin_of, slot_of=slot_of, T=T, cap=cap,
        n_bins=n_bins, tiles_per_core=tiles_per_core, node_src=node_src,
        rel=rel, invc=invc, s_tiles=s_tiles, s_per_core_pad=s_tiles * P,
        s_total=S_total,
    )


def _build_program(D, tpc, T, s_tiles, n_pos, repeat=1, loop=False):
    """Trace + compile the SPMD Bass program for the given config.

    repeat>1 re-executes the whole body (python-unrolled, or via a hardware
    For_i loop when loop=True) — used only for timing measurements.
    """
    f32 = mybir.dt.float32
    mm_dt = mybir.dt.float32r if MM_DTYPE == "f32r" else f32
    i32 = mybir.dt.int32
    ntiles = tpc * T

    nc = bacc.Bacc("TRN2", target_bir_lowering=False)
    x_in = nc.declare_dram_parameter("x", [tpc * T * P, D], mm_dt, isOutput=False)
    h_in = nc.declare_dram_parameter("h", [tpc * T * P, D], mm_dt, isOutput=False)
    rel_in = nc.declare_dram_parameter("relmap", [P, ntiles], f32, isOutput=False)
    invc_in = nc.declare_dram_parameter("invc", [P, tpc], f32, isOutput=False)
    iota_in = nc.declare_dram_parameter("iota", [P, P], f32, isOutput=False)
    pos_in = nc.declare_dram_parameter("pos", [n_pos, 3], f32, isOutput=False)
    soff_in = nc.declare_dram_parameter("soff", [P, s_tiles], i32, isOutput=False)

    xp_out = nc.declare_dram_parameter("xp", [tpc * P, D], f32, isOutput=True)
    hp_out = nc.declare_dram_parameter("hp", [tpc * P, D], f32, isOutput=True)
    ps_out = nc.declare_dram_parameter("ps", [s_tiles * P, 3], f32, isOutput=True)

    x_view = x_in[:, :].rearrange("(t p) d -> t p d", p=P)
    h_view = h_in[:, :].rearrange("(t p) d -> t p d", p=P)
    ps_view = ps_out[:, :].rearrange("(k p) e -> p k e", p=P)

    with tile.TileContext(nc) as tc:
        with (
            tc.tile_pool(name="const", bufs=1) as cpool,
            tc.tile_pool(name="io", bufs=3) as iopool,
            tc.tile_pool(name="s", bufs=6) as spool,
            tc.tile_pool(name="out", bufs=4) as opool,
            tc.tile_pool(name="psum", bufs=4, space="PSUM") as psum,
        ):
            iot = cpool.tile([P, P], f32)
            rmt = cpool.tile([P, ntiles], f32)
            ict = cpool.tile([P, tpc], f32)
            sot = cpool.tile([P, s_tiles], i32)
            nc.sync.dma_start(out=iot[:], in_=iota_in[:, :])
            nc.sync.dma_start(out=rmt[:], in_=rel_in[:, :])
            nc.sync.dma_start(out=ict[:], in_=invc_in[:, :])
            nc.sync.dma_start(out=sot[:], in_=soff_in[:, :])

            def body():
                # pos gather: 128 rows per indirect call
                pos_sb = cpool.tile([P, s_tiles * 3], f32, tag="pos_sb")
                for k in range(s_tiles):
                    nc.gpsimd.indirect_dma_start(
                        out=pos_sb[:, k * 3:(k + 1) * 3],
                        out_offset=None,
                        in_=pos_in[:, :],
                        in_offset=bass.IndirectOffsetOnAxis(
                            ap=sot[:, k:k + 1], axis=0),
                    )
                nc.sync.dma_start(
                    out=ps_view,
                    in_=pos_sb[:].rearrange("p (k e) -> p k e", e=3),
                )
                for t in range(tpc):
                    xt = iopool.tile([P, T * D], mm_dt, tag="xt")
                    ht = iopool.tile([P, T * D], mm_dt, tag="ht")
                    nc.sync.dma_start(
                        out=xt[:].rearrange("p (j d) -> p j d", j=T),
                        in_=x_view[t * T:(t + 1) * T].rearrange("t p d -> p t d"),
                    )
                    nc.sync.dma_start(
                        out=ht[:].rearrange("p (j d) -> p j d", j=T),
                        in_=h_view[t * T:(t + 1) * T].rearrange("t p d -> p t d"),
                    )
                    psx = psum.tile([P, D], f32, tag="psx")
                    psh = psum.tile([P, D], f32, tag="psh")
                    for j in range(T):
                        S = spool.tile([P, P], mm_dt, tag="S")
                        nc.vector.tensor_scalar(
                            out=S[:], in0=iot[:],
                            scalar1=rmt[:, t * T + j:t * T + j + 1], scalar2=None,
                            op0=mybir.AluOpType.is_equal,
                        )
                        nc.tensor.matmul(
                            out=psx[:], lhsT=S[:], rhs=xt[:, j * D:(j + 1) * D],
                            start=(j == 0), stop=(j == T - 1),
                        )
                        nc.tensor.matmul(
                            out=psh[:], lhsT=S[:], rhs=ht[:, j * D:(j + 1) * D],
                            start=(j == 0), stop=(j == T - 1),
                        )
                    ox = opool.tile([P, D], f32, tag="ox")
                    oh = opool.tile([P, D], f32, tag="oh")
                    nc.vector.tensor_scalar(
                        out=ox[:], in0=psx[:], scalar1=ict[:, t:t + 1],
                        scalar2=None, op0=mybir.AluOpType.mult,
                    )
                    nc.vector.tensor_scalar(
                        out=oh[:], in0=psh[:], scalar1=ict[:, t:t + 1],
                        scalar2=None, op0=mybir.AluOpType.mult,
                    )
                    nc.sync.dma_start(out=xp_out[t * P:(t + 1) * P, :], in_=ox[:])
                    nc.sync.dma_start(out=hp_out[t * P:(t + 1) * P, :], in_=oh[:])

            if loop and repeat > 1:
                with tc.For_i(0, repeat, 1):
                    body()
            else:
                for _ in range(repeat):
                    body()

    nc.compile()
    return nc


def _get_program(key):
    prog = _PROGRAM_CACHE.get(key)
    if prog is None:
        prog = _build_program(*key)
        _PROGRAM_CACHE[key] = prog
    return prog


# ----------------------------------------------------------------------------
# Entry point
# ----------------------------------------------------------------------------

def kernel(x, h, pos, cluster_map, sample_index, num_clusters):
    x = np.ascontiguousarray(np.asarray(x, dtype=np.float32))
    h = np.ascontiguousarray(np.asarray(h, dtype=np.float32))
    pos = np.ascontiguousarray(np.asarray(pos, dtype=np.float32))
    cluster_map = np.asarray(cluster_map).astype(np.int64)
    sample_index_in = np.asarray(sample_index)
    out_idx_dtype = sample_index_in.dtype  # unused; outputs are float
    sample_index = sample_index_in.astype(np.int64)
    C = int(num_clusters)
    N, D = x.shape

    L = _build_layout(cluster_map, sample_index, C)
    tpc, T, cap, n_bins = L["tiles_per_core"], L["T"], L["cap"], L["n_bins"]
    st = L["s_tiles"]
    spc = L["s_per_core_pad"]

    # per-core shards
    node_src = L["node_src"]
    valid = node_src >= 0
    x_packed = np.zeros((n_bins * cap, D), dtype=np.float32)
    h_packed = np.zeros((n_bins * cap, D), dtype=np.float32)
    x_packed[valid] = x[node_src[valid]]
    h_packed[valid] = h[node_src[valid]]

    soff_flat = np.zeros(N_CORES * spc, dtype=np.int32)
    soff_flat[:L["s_total"]] = sample_index.astype(np.int32)

    iota = np.broadcast_to(
        np.arange(P, dtype=np.float32)[None, :], (P, P)
    ).copy()

    in_maps = []
    rows_per_core = tpc * cap
    ntiles = tpc * T
    for c in range(N_CORES):
        r0 = c * rows_per_core
        rm = L["rel"][r0:r0 + rows_per_core].reshape(ntiles, P).T.copy()
        ic = L["invc"][c * tpc:(c + 1) * tpc].T.copy()
        so = soff_flat[c * spc:(c + 1) * spc].reshape(st, P).T.copy()
        in_maps.append({
            "x": x_packed[r0:r0 + rows_per_core],
            "h": h_packed[r0:r0 + rows_per_core],
            "relmap": rm,
            "invc": ic,
            "iota": iota,
            "pos": pos,
            "soff": so,
        })

    global _last_in_maps
    _last_in_maps = in_maps

    prog = _get_program((D, tpc, T, st, N, 1, False))
    res = run_bass_kernel_spmd(prog, in_maps, core_ids=list(range(N_CORES)))

    xp = np.concatenate([r["xp"] for r in res.results], axis=0)
    hp = np.concatenate([r["hp"] for r in res.results], axis=0)

    out_row = L["bin_of"] * P + L["slot_of"]
    x_pooled = xp[out_row]
    h_pooled = hp[out_row]

    # pos: core c holds padded rows [c*spc, (c+1)*spc); valid prefix varies
    pos_parts = []
    s_total = L["s_total"]
    for c in range(N_CORES):
        lo = c * spc
        n_valid = max(0, min(spc, s_total - lo))
        if n_valid:
            pos_parts.append(res.results[c]["ps"][:n_valid])
    pos_sampled = np.concatenate(pos_parts, axis=0)

    return (x_pooled, h_pooled, pos_sampled)
